# revision 30
# baseline (speedup 1.0000x reference)
"""Trainium2 Bass kernel: anchor classification labels via IoU >= 0.5 vs gt boxes.

Problem: anchorss (8, 262144, 4) [yc, xc, h, w]; gt_bboxess (8, 64, 4)
[y1, x1, y2, x2]; gt_counts (8, 1). Output labels (8, 262144, 1) int32 --
1 iff any valid gt has IoU >= 0.5 with the anchor.

Device algorithm (f32, division-free; x-coords pre-scaled by 3 on host so
the test reads  dy * dx3 - G >= S  where dx3 = 3*dx).  Per (gt, run):
    dy  = relu(min(y2, gy2)  - max(y1, gy1))     [DVE custom COVL]
    dx3 = relu(min(x23, gx23) - max(x13, gx13))  [DVE custom COVL]
    q   = dy * dx3 - G                           [DVE custom WSUBG]
    acc = max(acc, q)                            [DVE tensor_tensor]
  label = (acc >= S)                             [DVE stt is_ge -> u8]
All four per-item ops run on the DVE: same-engine in-order dependencies
need no semaphores, and measured hardware runs custom/TT/STT uniformly at
~0.93-1.03 ns/col with near-full pipelining, while any cross-engine hop
(Pool mult etc.) costs semaphore instructions and pipeline drains that
far outweigh the offload (Pool also only accepts fully dense operands).
Emission is software-pipelined one item ahead so no instruction depends
on its immediate predecessor.

Sharding: anchors of each batch are sorted by area S on the host; a gt's
area window [G/2, 2G] is then one contiguous run, column-tightened by a
per-column necessary bound.  All 8 batches are laid on one global sorted
axis and cut into 8 contiguous ranges at equal predicted-DVE-cost points
(item fixed cost charged at its first column, col span capped for DMA),
so each gt run lands on the few cores holding its columns (~1.3 runs per
gt) and per-core work is equal by construction.  Per-core programs are 8
arms of a tc.Switch on the core id (loaded via a tiny SBUF-staged DMA;
direct DRAM register loads cost ~10us).  Host ships 5 derived planes
(y1, y2, 3*x1, 3*x2, S) in per-core layout, DMA'd in arrival-ordered
chunks on 3 queues (sync/scalar/gpsimd); items are emitted in data-
arrival order; labels ship back per segment as u8 and the host scatters
them to the original order.  Rounding differences vs the reference chain
are ~1 ulp; the measured minimum |3*inter-(S+G)|/(S+G) margin is 7.8e-6,
so they cannot flip a label (verified: 0 mismatches).
"""

import os
import sys

os.environ.setdefault("MYCRO_LOCAL_CACHE", "1")
if "/opt/trn_rl_repo" not in sys.path:
    sys.path.insert(0, "/opt/trn_rl_repo")

import numpy as np

import concourse.bacc as bacc
import concourse.mybir as mybir
import concourse.tile as tile
import concourse.dve_ops as dve_ops
from concourse.dve_spec import (
    Spec, Src0, Src1, C0, C1, lower, relu, minn, maxx, _has_src1,
)
from concourse.dve_uop import DveOpSpec
from concourse.bass_utils import run_bass_kernel_spmd

B, N, A = 8, 262144, 64
P = 128
NCORES = 8
COLS = N // P                 # 2048 sorted rank-columns per batch
NSEG = 2                      # segments per batch
DT = mybir.dt.float32
U8 = mybir.dt.uint8
GUARD = 1e-5
NEG_INIT = -1e30
mm = mybir.AluOpType
ET = mybir.EngineType


def _register_op(name, spec):
    for op in dve_ops.OPS:
        if op.name == name:
            return op
    row = dve_ops._CUSTOM_DVE_ROW_BASE + len(dve_ops.OPS)
    shas = {}
    for ver in ("v3", "v4"):
        try:
            uops = lower(spec, ver=ver)
            shas[ver] = DveOpSpec(
                name=name, opcode=row, uops=uops, rd1_en=_has_src1(spec)
            ).sha(ver)
        except Exception:
            pass
    op = dve_ops.DveOp(name, spec, subdim=False, uops_sha=shas)
    dve_ops.OPS.append(op)
    dve_ops._SUB_OPCODE_FOR_NAME[name] = row
    dve_ops.CUSTOM_DVE_SPECS[name] = spec
    return op


# out = relu(min(in0, s0) - max(in1, s1))  -- 1-D interval overlap
COVL = _register_op("ANT_COVL", Spec(
    body=relu(minn(Src0, C0) - maxx(Src1, C1)),
    reference=lambda in0, in1, s0, s1, imm2: np.maximum(
        np.minimum(in0, s0) - np.maximum(in1, s1), 0.0
    ).astype(np.float32),
))
# out = in0 * in1 - s0  -- fused pair score (dy * dx3 - G)
WSUBG = _register_op("ANT_WSUBG", Spec(
    body=Src0 * Src1 - C0,
    reference=lambda in0, in1, s0, s1, imm2: (in0 * in1 - s0).astype(np.float32),
))


def _prepare(anchorss, gt_bboxess, gt_counts):
    """Sort by area per batch, build gt runs, cut segments, assign to cores."""
    f32 = np.float32
    a = np.asarray(anchorss, f32)
    g = np.asarray(gt_bboxess, f32)
    cnts = np.asarray(gt_counts).reshape(-1)

    batch = []
    for b in range(B):
        y, x, h, w = a[b, :, 0], a[b, :, 1], a[b, :, 2], a[b, :, 3]
        s_key = (h * w).astype(f32)
        perm = np.argsort(s_key, kind="stable")
        y1 = (y - h * f32(0.5)).astype(f32)
        y2 = (y1 + h).astype(f32)
        x1 = (x - w * f32(0.5)).astype(f32)
        x2 = (x1 + w).astype(f32)
        planes = {
            "y1": y1[perm], "y2": y2[perm],
            "x13": (f32(3.0) * x1).astype(f32)[perm],
            "x23": (f32(3.0) * x2).astype(f32)[perm],
            "s": s_key[perm],
        }
        s_sorted = s_key[perm]
        hs = h[perm].reshape(COLS, P)
        ws = w[perm].reshape(COLS, P)
        hmax = hs.max(1).astype(np.float64)
        wmax = ws.max(1).astype(np.float64)
        smin = s_sorted.reshape(COLS, P).min(1).astype(np.float64)
        gy1a, gx1a, gy2a, gx2a = g[b, :, 0], g[b, :, 1], g[b, :, 2], g[b, :, 3]
        Ga = (np.float32(gy2a - gy1a) * np.float32(gx2a - gx1a)).astype(f32)
        items = []
        for ai in range(int(cnts[b])):
            Gv = float(Ga[ai])
            glo = int(np.searchsorted(s_sorted, Gv * 0.5 * (1 - GUARD), side="left"))
            ghi = int(np.searchsorted(s_sorted, Gv * 2.0 * (1 + GUARD), side="right"))
            if ghi <= glo:
                continue
            lo = glo // P
            hi = -(-ghi // P)
            gh = float(gy2a[ai] - gy1a[ai])
            gw = float(gx2a[ai] - gx1a[ai])
            ub = (3.0 * np.minimum(hmax[lo:hi], gh) * np.minimum(wmax[lo:hi], gw)
                  - smin[lo:hi] - Gv)
            alive = ub >= -(GUARD * (smin[lo:hi] + Gv) + 1e-9)
            if not alive.any():
                continue
            nz = np.nonzero(alive)[0]
            lo, hi = lo + int(nz[0]), lo + int(nz[-1]) + 1
            items.append((lo, hi, float(gy1a[ai]), float(gy2a[ai]),
                          float(f32(3.0) * f32(gx1a[ai])),
                          float(f32(3.0) * f32(gx2a[ai])), Gv))
        batch.append(dict(planes=planes, items=items, perm=perm))

    # one global equal-cost 8-way cut: concatenate batches on one axis,
    # prefix-sum the predicted per-column DVE cost (item fixed charged at
    # the item's first column), cut at octiles, snap to 8-col alignment
    def _aligned_items(b, lo_s, hi_s):
        out = []
        for (lo, hi, gy1v, gy2v, gx13v, gx23v, Gv) in batch[b]["items"]:
            ov_lo, ov_hi = max(lo, lo_s), min(hi, hi_s)
            if ov_hi > ov_lo:
                ov_lo = max(lo_s, (ov_lo // 8) * 8)
                ov_hi = min(hi_s, -(-ov_hi // 8) * 8)
                out.append((ov_lo, ov_hi, gy1v, gy2v, gx13v, gx23v, Gv))
        return out

    mc = np.full(B * COLS, 1.25)          # finalize slope
    for b in range(B):
        for (lo, hi, *_r) in batch[b]["items"]:
            mc[b * COLS + lo:b * COLS + hi] += 3.82
            mc[b * COLS + lo] += 700.0     # item fixed at first col
    cum = np.concatenate([[0.0], np.cumsum(mc)])
    cuts = [0]
    for k in range(1, NCORES):
        pos = int(np.searchsorted(cum, cum[-1] * k / NCORES))
        cuts.append(min(B * COLS - 8, (pos // 8) * 8))
    cuts.append(B * COLS)
    # cap each core's column span (DMA bytes scale with the max span)
    CAP = 2600
    for k in range(1, NCORES + 1):
        if cuts[k] - cuts[k - 1] > CAP:
            cuts[k] = cuts[k - 1] + CAP
    for k in range(NCORES - 1, 0, -1):   # fix tail overflow backwards
        if cuts[k + 1] - cuts[k] > CAP:
            cuts[k] = cuts[k + 1] - CAP

    cores = []
    for c in range(NCORES):
        glo, ghi = cuts[c], cuts[c + 1]
        seg_list = []
        b0, b1 = glo // COLS, (ghi - 1) // COLS
        for b in range(b0, b1 + 1):
            lo_s = max(glo - b * COLS, 0)
            hi_s = min(ghi - b * COLS, COLS)
            if hi_s > lo_s:
                seg_list.append((b, lo_s, hi_s, _aligned_items(b, lo_s, hi_s)))
        cores.append(dict(segs=seg_list,
                          cols=sum(s[2] - s[1] for s in seg_list)))
    FD = max(c["cols"] for c in cores)

    plans, fields, scatter = [], [], []
    for c in cores:
        off = 0
        seg_list, scat_c = [], []
        pl = {k: np.zeros((P, FD), f32) for k in ("y1", "y2", "x13", "x23", "s")}
        for (b, lo_s, hi_s, its) in c["segs"]:
            width = hi_s - lo_s
            for k in pl:
                blk = batch[b]["planes"][k][lo_s * P:hi_s * P].reshape(width, P).T
                pl[k][:, off:off + width] = blk
            seg_items = [(off + lo - lo_s, off + hi - lo_s,
                          gy1v, gy2v, gx13v, gx23v, Gv)
                         for (lo, hi, gy1v, gy2v, gx13v, gx23v, Gv) in its]
            # order by data arrival: an item can start once the DMA chunk
            # holding its last column has landed
            seg_items.sort(key=lambda it: it[1])
            seg_list.append(dict(off=off, width=width, items=seg_items))
            scat_c.append((b, lo_s, hi_s, off))
            off += width
        plans.append(seg_list)
        fields.append(pl)
        scatter.append(scat_c)
    perms = [batch[b]["perm"] for b in range(B)]
    return plans, FD, fields, scatter, perms


def build_nc(plans, FD):
    nc = bacc.Bacc(None, target_bir_lowering=False)
    ins = {}
    for f in ("y1", "y2", "x13", "x23", "s"):
        ins[f] = nc.declare_dram_parameter(f, [P, FD], DT, isOutput=False)
    outp = nc.declare_dram_parameter("out", [P, FD], U8, isOutput=True)

    WMAX = max((it[1] - it[0]) for segl in plans for sg in segl for it in sg["items"])
    SEGMAX = max(sg["width"] for segl in plans for sg in segl)
    NACC = max(len(segl) for segl in plans)

    with tile.TileContext(nc) as tc:
        with tc.tile_pool(name="pers", bufs=1) as pers, \
             tc.tile_pool(name="work", bufs=8) as work, \
             tc.tile_pool(name="qp", bufs=6) as qp:
            t = {f: pers.tile([P, FD], DT, tag=f, name=f"t_{f}") for f in ins}
            accs = [pers.tile([P, SEGMAX], DT, tag=f"acc{i}", name=f"acc{i}")
                    for i in range(NACC)]
            lb = pers.tile([P, FD], U8, tag="lb")
            for i in range(NACC):
                nc.gpsimd.memset(accs[i][:], NEG_INIT)

            # partition id: DMA the [1,1] DRAM tensor into SBUF once, then
            # cheap per-engine register loads (direct DRAM reg loads cost
            # ~10us of engine time at startup)
            pid_sb = pers.tile([1, 1], mybir.dt.uint32, tag="pid", name="pid_sb")
            nc.gpsimd.dma_start(out=pid_sb[:], in_=nc.partition_id_tensor[0:1, 0:1])

            # chunked input DMAs, first chunks first
            NCH = 8
            bounds = [(FD * i // NCH) // 8 * 8 for i in range(NCH + 1)]
            bounds[-1] = FD
            engs = [nc.sync, nc.scalar, nc.gpsimd]
            k = 0
            # compute planes first (chunk-major); the s plane is only read
            # by the finalizes, so its chunks ship after everything else
            for ch in range(len(bounds) - 1):
                cs = slice(bounds[ch], bounds[ch + 1])
                for f in ("y1", "y2", "x13", "x23"):
                    engs[k % len(engs)].dma_start(out=t[f][:, cs], in_=ins[f][:, cs])
                    k += 1
            for ch in range(len(bounds) - 1):
                cs = slice(bounds[ch], bounds[ch + 1])
                engs[k % len(engs)].dma_start(out=t["s"][:, cs], in_=ins["s"][:, cs])
                k += 1

            index = {}
            for et, eng in ((ET.DVE, nc.vector),):
                tmp = eng.alloc_register(f"pid_{et.name}")
                eng.reg_load(tmp, pid_sb[0:1, 0:1])
                index[et] = eng.snap(tmp, donate=True, min_val=0,
                                     max_val=NCORES - 1)
            for c in tc.Switch(index, NCORES):
                for si, sg in enumerate(plans[c]):
                    acc = accs[si]
                    # all-DVE, software-pipelined two items deep: WSUBG+fold
                    # of item i-2 run between the COVL pairs of items i-1/i,
                    # so every instruction is >=4 slots from its producers
                    pend = []
                    LAG = 2

                    def _flush(force=False):
                        while pend and (force or len(pend) > LAG):
                            (psl, pdy, pdx, pG) = pend.pop(0)
                            q = qp.tile([P, pdy.shape[1]], DT, tag="q", name="q")
                            nc.vector._custom_dve(
                                WSUBG, out=q[:], in0=pdy[:], in1=pdx[:], s0=pG)
                            nc.vector.tensor_tensor(
                                out=acc[:, psl], in0=q[:], in1=acc[:, psl],
                                op=mm.max)

                    for (lo, hi, gy1v, gy2v, gx13v, gx23v, Gv) in sg["items"]:
                        wd = hi - lo
                        sl = slice(lo, hi)
                        rsl = slice(lo - sg["off"], hi - sg["off"])
                        dy = work.tile([P, wd], DT, tag="dy", name="dy")
                        dx = work.tile([P, wd], DT, tag="dx", name="dx")
                        nc.vector._custom_dve(
                            COVL, out=dy[:], in0=t["y2"][:, sl],
                            in1=t["y1"][:, sl], s0=gy2v, s1=gy1v)
                        nc.vector._custom_dve(
                            COVL, out=dx[:], in0=t["x23"][:, sl],
                            in1=t["x13"][:, sl], s0=gx13v if False else gx23v, s1=gx13v)
                        pend.append((rsl, dy, dx, Gv))
                        _flush()
                    _flush(force=True)
                    # finalize this segment's labels on DVE (u8 out, stt mode)
                    ss = slice(sg["off"], sg["off"] + sg["width"])
                    nc.vector.scalar_tensor_tensor(
                        out=lb[:, ss], in0=acc[:, :sg["width"]], scalar=0.0,
                        in1=t["s"][:, ss], op0=mm.subtract, op1=mm.is_ge)
            nc.sync.dma_start(out=outp[:], in_=lb[:])
    nc.compile()
    return nc


_CACHE = {}


def _run(anchorss, gt_bboxess, gt_counts, use_anchor, trace=False):
    assert int(np.asarray(use_anchor)) == 1
    plans, FD, fields, scatter, perms = _prepare(anchorss, gt_bboxess, gt_counts)

    key = (FD,) + tuple(
        tuple((sg["off"], sg["width"], tuple(sg["items"])) for sg in segl)
        for segl in plans)
    if _CACHE.get("key") != key:
        _CACHE["nc"] = build_nc(plans, FD)
        _CACHE["key"] = key
    nc = _CACHE["nc"]

    in_maps = [
        {f: np.ascontiguousarray(fields[c][f]) for f in fields[c]}
        for c in range(NCORES)
    ]
    res = run_bass_kernel_spmd(nc, in_maps, core_ids=list(range(NCORES)), trace=trace)

    out = np.empty((B, N, 1), np.int32)
    lab_sorted = [np.empty(N, np.int32) for _ in range(B)]
    for c in range(NCORES):
        labc = np.asarray(res.results[c]["out"])  # [P, FD] u8
        for (b, lo_s, hi_s, off) in scatter[c]:
            width = hi_s - lo_s
            blk = labc[:, off:off + width]        # [P, width]
            lab_sorted[b][lo_s * P:hi_s * P] = blk.T.reshape(width * P)
    for b in range(B):
        out[b, perms[b], 0] = lab_sorted[b]
    return out, res


def kernel(anchorss, gt_bboxess, gt_counts, use_anchor=1):
    out, _ = _run(anchorss, gt_bboxess, gt_counts, use_anchor, trace=False)
    return out


def kernel_traced(anchorss, gt_bboxess, gt_counts, use_anchor=1):
    return _run(anchorss, gt_bboxess, gt_counts, use_anchor, trace=True)
